# revision 12
# baseline (speedup 1.0000x reference)
"""Trainium2 Bass kernel for nn_DMRI2INetworkLayer (additive-attention pooling).

Reference (per batch row b):
    pre[s,h]  = X_item[b]@Wc + X_series[b,s]@We + pos[s]@Wp
    scores[s] = sum_h z[h]*tanh(pre[s,h])
    score_sum = sum_s where(mask, scores, 0)
    attn      = softmax(where(mask, scores, -inf))
    out[b]    = concat(sum_s attn[s]*X_series[b,s], score_sum)

Sharding: data-parallel over batch B=4096 across 8 NeuronCores (512 rows
per core).

Device design (v2), per core bc=512 = 4 bt-blocks of 128 = 16 tb of 32:
  - pos term folded into X on host: X' = X + pos@Wp@pinv(We) (exact:
    pinv(We)@We = I since We has full column rank 64). xt = X'^T bf16
    [128d, bc*200] streams as matmul rhs with lhsT=[We|We] in two 64-col
    PE groups -> pre PSUM [128(2x64h), 400] per 4-b group.
  - item term c = X_item@Wc folded in as the per-partition ACT bias of
    tanh (2 ACTs of [128,200] per group, bias col = [c_bA; c_bB]).
  - z-dot: per (group, col-half) matmul with a zero-padded 32-col z
    pattern stationary writes scores DIRECTLY into a dense per-bt
    [128b, 200s] PSUM tile (no scatter DMAs).
  - masked softmax without max-subtraction (|scores| <= ||z||_1 ~ 5):
    exp on ACT, mask-mult + den/ssum via fused scalar_tensor_tensor
    accum_out on DVE.
  - weighted sum: per-(b, s-chunk) matmuls, X s-major slice stationary
    (xn0: s 0:128; xn1: s 128:200 with pad rows zeroed by GPSIMD),
    attn^T column rhs, accumulated into one PSUM bank [128d, 512b].
  - output: PE transpose [d,b]->[b,d]; no batch permutation anywhere.
"""
import os
import sys

sys.path.insert(0, "/opt/trn_rl_repo")

DEBUG = int(os.environ.get("K_DEBUG", "0"))

import numpy as np
import ml_dtypes
from contextlib import ExitStack

import concourse.bass as bass
import concourse.bacc as bacc
import concourse.tile as tile
from concourse import mybir
from concourse.bass_utils import run_bass_kernel_spmd

BF = mybir.dt.bfloat16
F32 = mybir.dt.float32
BF_NP = ml_dtypes.bfloat16

N_CORES = 8
B, S, D, H = 4096, 200, 128, 64
BC = B // N_CORES          # batch rows per core
NBT = 4                    # 128-b blocks per core
NTB = 4                    # 32-b tiles per block

_CACHE = {}


def build_nc(bc=BC):
    if bc in _CACHE:
        return _CACHE[bc]
    nbt = bc // 128
    nc = bacc.Bacc("TRN2", target_bir_lowering=False, num_devices=N_CORES)

    xt = nc.declare_dram_parameter("xt", [D, bc * S], BF, isOutput=False)
    xn0 = nc.declare_dram_parameter("xn0", [128, bc * D], BF, isOutput=False)
    xn1 = nc.declare_dram_parameter("xn1", [72, bc * D], BF, isOutput=False)
    wew2 = nc.declare_dram_parameter("wew2", [D, 128], BF, isOutput=False)
    wcw2 = nc.declare_dram_parameter("wcw2", [D, 128], BF, isOutput=False)
    xitT = nc.declare_dram_parameter("xitT", [D, bc], BF, isOutput=False)
    zzpat = nc.declare_dram_parameter("zzpat", [128, 512], BF, isOutput=False)
    m01p = nc.declare_dram_parameter("m01p", [bc, S], F32, isOutput=False)
    idbf = nc.declare_dram_parameter("idbf", [128, 128], BF, isOutput=False)
    idf = nc.declare_dram_parameter("idf", [128, 128], F32, isOutput=False)
    out_attn = nc.declare_dram_parameter("out_attn", [bc, D], F32, isOutput=True)
    out_ssum = nc.declare_dram_parameter("out_ssum", [bc, 1], F32, isOutput=True)

    MULT = mybir.AluOpType.mult

    with tile.TileContext(nc) as tc, ExitStack() as ctx:
        const = ctx.enter_context(tc.tile_pool(name="const", bufs=1))
        xtp = ctx.enter_context(tc.tile_pool(name="xtp", bufs=2))
        xnp = ctx.enter_context(tc.tile_pool(name="xnp", bufs=6))
        thp = ctx.enter_context(tc.tile_pool(name="thp", bufs=6))
        smp = ctx.enter_context(tc.tile_pool(name="smp", bufs=2))
        atp = ctx.enter_context(tc.tile_pool(name="atp", bufs=2))
        outp = ctx.enter_context(tc.tile_pool(name="outp", bufs=2))
        pre_ps = ctx.enter_context(tc.tile_pool(name="pre_ps", bufs=4, space="PSUM"))
        sc_ps = ctx.enter_context(tc.tile_pool(name="sc_ps", bufs=2, space="PSUM"))
        o5_ps = ctx.enter_context(tc.tile_pool(name="o5_ps", bufs=1, space="PSUM"))
        t_ps = ctx.enter_context(tc.tile_pool(name="t_ps", bufs=1, space="PSUM"))

        # ---------- constants ----------
        def cdma(shape, dt_, src, tag):
            t = const.tile(shape, dt_, tag=tag, name=tag)
            nc.sync.dma_start(t[:], src)
            return t

        wew2_t = cdma([D, 128], BF, wew2[:], "wew2_t")
        wcw2_t = cdma([D, 128], BF, wcw2[:], "wcw2_t")
        xitT_t = cdma([D, bc], BF, xitT[:], "xitT_t")
        zzpat_t = cdma([128, 512], BF, zzpat[:], "zzpat_t")
        idbf_t = cdma([128, 128], BF, idbf[:], "idbf_t")
        idf_t = cdma([128, 128], F32, idf[:], "idf_t")
        zero_t = const.tile([128, 128], BF, tag="zero_t", name="zero_t")
        nc.vector.memset(zero_t[:], 0.0)
        m01_t = []
        for bt in range(nbt):
            t = const.tile([128, S], F32, tag=f"m01_{bt}", name=f"m01_{bt}")
            nc.sync.dma_start(t[:], m01p[bt * 128:(bt + 1) * 128, :])
            m01_t.append(t)

        # ---------- c = X_item@Wc -> ctb bias table [128, bc/2] ----------
        c_ps = o5_ps.tile([128, bc], F32, tag="o5", name="c_ps")
        nc.tensor.matmul(c_ps[:], wcw2_t[:], xitT_t[:], start=True, stop=True,
                         skip_group_check=True)
        ctb = const.tile([128, bc // 2], F32, tag="ctb", name="ctb")
        # ctb col (16tb+2g+c): rows 0:64 = c[32tb+4g+c], rows 64:128 = c[32tb+4g+2+c]
        src_top = c_ps[0:64, :].rearrange("p (t g h c) -> p t g h c",
                                          g=8, h=2, c=2)[:, :, :, 0, :]
        src_bot = c_ps[64:128, :].rearrange("p (t g h c) -> p t g h c",
                                            g=8, h=2, c=2)[:, :, :, 1, :]
        dst_top = ctb[0:64, :].rearrange("p (t g c) -> p t g c", g=8, c=2)
        dst_bot = ctb[64:128, :].rearrange("p (t g c) -> p t g c", g=8, c=2)
        nc.vector.tensor_copy(dst_top, src_top)
        nc.vector.tensor_copy(dst_bot, src_bot)

        # ---------- weighted-sum accumulator ----------
        o5 = o5_ps.tile([128, bc], F32, tag="o5", name="o5")
        nc.vector.memset(o5[:], 0.0)
        n_mm5 = 0

        for bt in range(nbt):
            sc_t = sc_ps.tile([128, S], F32, tag="sc", name=f"sc_{bt}")
            # Full-region zero matmul: zeroes data AND sets every
            # has_written bit (ACT-Copy/DVE PSUM reads are bit-gated).
            # The zz matmuls below accumulate onto it with start=False.
            nc.tensor.matmul(sc_t[:], zero_t[:], zzpat_t[:, 0:S],
                             start=True, stop=True, skip_group_check=True)
            xn_tiles = []
            for q in range(NTB):
                tb = bt * NTB + q
                xt_t = xtp.tile([D, 32 * S], BF, tag="xt", name=f"xt_{tb}")
                nc.sync.dma_start(xt_t[:], xt[:, tb * 32 * S:(tb + 1) * 32 * S])
                xn0_t = xnp.tile([128, 32 * D], BF, tag="xn0", name=f"xn0_{tb}")
                nc.sync.dma_start(xn0_t[:], xn0[:, tb * 32 * D:(tb + 1) * 32 * D])
                xn1_t = xnp.tile([128, 32 * D], BF, tag="xn1", name=f"xn1_{tb}")
                nc.gpsimd.memset(xn1_t[64:128, :], 0.0)
                nc.sync.dma_start(xn1_t[0:72, :], xn1[:, tb * 32 * D:(tb + 1) * 32 * D])
                xn_tiles.append((xn0_t, xn1_t))

                for g in range(8):
                    pre = pre_ps.tile([128, 2 * S], F32, tag="pre", name=f"pre{tb}_{g}")
                    cA = (4 * g) * S
                    nc.tensor.matmul(pre[0:64, :], wew2_t[:, 0:64],
                                     xt_t[:, cA:cA + 2 * S],
                                     start=True, stop=True, tile_position=(0, 0),
                                     skip_group_check=True)
                    nc.tensor.matmul(pre[64:128, :], wew2_t[:, 64:128],
                                     xt_t[:, cA + 2 * S:cA + 4 * S],
                                     start=True, stop=True, tile_position=(0, 64),
                                     skip_group_check=True)
                    th = thp.tile([128, 2 * S], BF, tag="th", name=f"th{tb}_{g}")
                    jc = 16 * tb + 2 * g
                    nc.scalar.activation(th[:, 0:S], pre[:, 0:S],
                                         mybir.ActivationFunctionType.Tanh,
                                         bias=ctb[:, jc:jc + 1])
                    nc.scalar.activation(th[:, S:2 * S], pre[:, S:2 * S],
                                         mybir.ActivationFunctionType.Tanh,
                                         bias=ctb[:, jc + 1:jc + 2])
                    for c in range(2):
                        zs = 32 * (2 * g + c)
                        nc.tensor.matmul(sc_t[32 * q:32 * q + 32, :],
                                         zzpat_t[:, zs:zs + 32],
                                         th[:, c * S:(c + 1) * S],
                                         start=False, stop=False,
                                         tile_position=(0, 32 * q),
                                         skip_group_check=True)

            # ---------- masked softmax over s (max-free; |scores|<~8) ----------
            ex = smp.tile([128, S], F32, tag="ex", name=f"ex_{bt}")
            nc.scalar.activation(ex[:], sc_t[:], mybir.ActivationFunctionType.Exp)
            exm = smp.tile([128, S], F32, tag="exm", name=f"exm_{bt}")
            den = smp.tile([128, 1], F32, tag="den", name=f"den_{bt}")
            nc.vector.scalar_tensor_tensor(exm[:], ex[:], 1.0, m01_t[bt][:],
                                           op0=MULT, op1=MULT, accum_out=den[:])
            # DVE reads of PSUM return 0 where has_written is clear (the
            # zero-padded zz stationaries only set bits on their 2 real
            # rows), so bounce scores through ACT (raw PSUM read) to SBUF.
            scs = smp.tile([128, S], F32, tag="scs", name=f"scs_{bt}")
            nc.scalar.activation(scs[:], sc_t[:], mybir.ActivationFunctionType.Copy)
            scm = smp.tile([128, S], F32, tag="scm", name=f"scm_{bt}")
            ssum = smp.tile([128, 1], F32, tag="ssum", name=f"ssum_{bt}")
            nc.vector.scalar_tensor_tensor(scm[:], scs[:], 1.0, m01_t[bt][:],
                                           op0=MULT, op1=MULT, accum_out=ssum[:])
            if DEBUG == 1:
                nc.sync.dma_start(out_ssum[bt * 128:(bt + 1) * 128, :], den[:])
                nc.sync.dma_start(out_attn[bt * 128:(bt + 1) * 128, :],
                                  scm[:, 0:D])
            else:
                nc.sync.dma_start(out_ssum[bt * 128:(bt + 1) * 128, :], ssum[:])
            rden = smp.tile([128, 1], F32, tag="rden", name=f"rden_{bt}")
            nc.vector.reciprocal(rden[:], den[:])
            attn = atp.tile([128, 256], BF, tag="attn", name=f"attn_{bt}")
            nc.vector.memset(attn[:, S:256], 0.0)
            nc.vector.tensor_scalar_mul(attn[:, 0:S], exm[:], rden[:])

            # ---------- attn^T ----------
            tp1 = t_ps.tile([128, 128], BF, tag="tp", name=f"tp1_{bt}")
            nc.tensor.transpose(tp1[:], attn[:, 0:128], idbf_t[:])
            atT_lo = atp.tile([128, 128], BF, tag="atT_lo", name=f"atTl_{bt}")
            nc.vector.tensor_copy(atT_lo[:], tp1[:])
            tp2 = t_ps.tile([128, 128], BF, tag="tp", name=f"tp2_{bt}")
            nc.tensor.transpose(tp2[:], attn[:, 128:256], idbf_t[:])
            atT_hi = atp.tile([128, 128], BF, tag="atT_hi", name=f"atTh_{bt}")
            nc.vector.tensor_copy(atT_hi[:], tp2[:])

            # ---------- weighted sum over s ----------
            for q in range(NTB):
                xn0_t, xn1_t = xn_tiles[q]
                for j in range(32):
                    col = 32 * q + j
                    b = 128 * bt + col
                    n_mm5 += 2
                    nc.tensor.matmul(o5[:, b:b + 1], xn0_t[:, j * D:(j + 1) * D],
                                     atT_lo[:, col:col + 1],
                                     start=False, stop=False, skip_group_check=True)
                    nc.tensor.matmul(o5[:, b:b + 1], xn1_t[:, j * D:(j + 1) * D],
                                     atT_hi[:, col:col + 1],
                                     start=False, stop=(n_mm5 == 2 * bc),
                                     skip_group_check=True)

        # ---------- drain weighted sum, transpose to [b, d], store ----------
        o5_s = outp.tile([D, bc], F32, tag="o5_s", name="o5_s")
        nc.vector.tensor_copy(o5_s[:], o5[:])
        for t in range(bc // 128):
            ot = t_ps.tile([128, 128], F32, tag="tp", name=f"ot_{t}")
            nc.tensor.transpose(ot[:], o5_s[:, t * 128:(t + 1) * 128], idf_t[:])
            ob = outp.tile([128, D], F32, tag="ob", name=f"ob_{t}")
            nc.vector.tensor_copy(ob[:], ot[:])
            if DEBUG != 1:
                nc.sync.dma_start(out_attn[t * 128:(t + 1) * 128, :], ob[:])

    nc.compile()
    _CACHE[bc] = nc
    return nc


def _prep_core(Xs, Xit, mask, pfold, We, Wp, Wc, z, bc):
    """Host-side marshalling for one core's shard. pfold = pos@Wp@pinv(We)."""
    d = {}
    Xp = Xs + pfold[None, :, :]
    d["xt"] = np.ascontiguousarray(Xp.transpose(2, 0, 1).reshape(D, bc * S)).astype(BF_NP)
    xn = Xs.transpose(1, 0, 2)                      # [S, bc, D]
    d["xn0"] = np.ascontiguousarray(xn[0:128].reshape(128, bc * D)).astype(BF_NP)
    d["xn1"] = np.ascontiguousarray(xn[128:S].reshape(72, bc * D)).astype(BF_NP)
    d["wew2"] = np.concatenate([We, We], 1).astype(BF_NP)
    d["wcw2"] = np.concatenate([Wc, Wc], 1).astype(BF_NP)
    d["xitT"] = np.ascontiguousarray(Xit.T).astype(BF_NP)
    zp = np.zeros((128, 512), np.float32)
    for g in range(8):
        for c in range(2):
            sl = 32 * (2 * g + c)
            zp[0:64, sl + 4 * g + c] = z
            zp[64:128, sl + 4 * g + 2 + c] = z
    d["zzpat"] = zp.astype(BF_NP)
    d["m01p"] = np.ascontiguousarray(mask.astype(np.float32))
    d["idbf"] = np.eye(128, dtype=np.float32).astype(BF_NP)
    d["idf"] = np.eye(128, dtype=np.float32)
    return d


def make_in_maps(X_series, pos_series, X_item, valid_mask, Wc, Wp, We, z, bc):
    We64 = np.asarray(We, np.float64)
    pfold = (np.asarray(pos_series, np.float64) @ np.asarray(Wp, np.float64)
             @ np.linalg.pinv(We64)).astype(np.float32)
    in_maps = []
    for k in range(N_CORES):
        sl = slice(k * bc, (k + 1) * bc)
        in_maps.append(_prep_core(np.asarray(X_series[sl], np.float32),
                                  np.asarray(X_item[sl], np.float32),
                                  np.asarray(valid_mask[sl]),
                                  pfold,
                                  np.asarray(We, np.float32),
                                  np.asarray(Wp, np.float32),
                                  np.asarray(Wc, np.float32),
                                  np.asarray(z, np.float32), bc))
    return in_maps


def _unshard(results, bc):
    outs = []
    for k in range(len(results)):
        outs.append(np.concatenate([results[k]["out_attn"],
                                    results[k]["out_ssum"]], axis=1))
    return np.concatenate(outs, axis=0)


def kernel(X_series, pos_series, X_item, valid_mask, Wc, Wp, We, z):
    X_series = np.asarray(X_series, np.float32)
    bc = X_series.shape[0] // N_CORES
    nc = build_nc(bc)
    in_maps = make_in_maps(X_series, pos_series, X_item, valid_mask,
                           Wc, Wp, We, z, bc)
    res = run_bass_kernel_spmd(nc, in_maps, list(range(N_CORES)))
    return _unshard(res.results, bc)
